# revision 46
# baseline (speedup 1.0000x reference)
"""Stress-majorization loss kernel for Trainium2 (8 NeuronCores).

Problem: pos [8192,2] f32, dist [8192,8192] f32 ->
    scalar sum of ((|p_i - p_j| - d_ij)/d_ij)^2 over entries with d_ij != 0.

Strategy (per-core row sharding, 1024 rows each):
 - Host: ship r = 1/d in bf16 (r = 0 for d==0 entries -> each contributes
   exactly (0-1)^2 = 1, removed via the host-side zero count).  Halves HBM
   traffic vs shipping d (16MB/core) and removes the device reciprocal
   pass.  Squared pairwise distances factor as a K=24 bf16 matmul.

 - Device facts driving the design (all measured on this part):
     * PE clock is pinned at 1.2 GHz (HAM never leaves K=4/8 regardless
       of matmul stream density) -> sq matmuls alone cost ~61.5us.
     * ACT is 1x rate, dtype-independent -> any full pass ~57-64us.
     * DVE: tensor_tensor 2x bf16 (~40us/pass), tensor_scalar 4x
       (~23us/pass), scalar_tensor_tensor 1x (~68us/pass, accum works).
     * tensor_tensor_reduce crashes the device - do not use.

 - Per row-tile [128, 8192], 6 chunks [1408,1536,1536,1408,1536,768]
   alternating two 3-bank PSUM pools globally (even chunk count per
   row-tile -> no same-pool seam at row-tile boundaries, which would
   head-of-line-block the in-order PE) + 1 gram bank.  Every chunk:
   PE sq matmul -> ACT sqrt(psum) -> pred bf16 -> DVE TT w = pred*r
   (in place, 2x).  Then one of three reduce classes per chunk, chosen
   to balance ACT/DVE/PE at ~81us each:
     G: DVE TS (v=w-1, 4x) + PE gram v_blk^T v_blk accumulated into
        one PSUM bank across the kernel; host reads trace(gram).
     A: ACT Square(w, bias=-1, accum_out) direct sum((w-1)^2).
     S: DVE STT (w-2)*w with accum_out = sum(w^2-2w); host adds M.
   The last row-tile runs gram-free so the PE flush doesn't extend the
   tail; the first row-tile uses per-chunk b/r DMAs to fill the pipe.
 - Host: combine per-class accumulators, minus zero count.
"""
import sys
sys.path.insert(0, "/opt/trn_rl_repo")

import numpy as np
import ml_dtypes

N = 8192
NCORES = 8
ROWS_PER_CORE = N // NCORES          # 1024
RTILES = ROWS_PER_CORE // 128        # 8 row tiles of 128
_WIDTHS = [1408, 1536, 1536, 1408, 1536, 768]
CHUNKS = []
_off = 0
for _w in _WIDTHS:
    CHUNKS.append((_off, _w))
    _off += _w
assert _off == N
NCH = len(CHUNKS)
GRAM_LAG = 3                         # emit gram MMs this many chunks behind
KB = 4                               # base contraction dim
NPAIR = 6                            # bf16 split term-pairs kept
K = KB * NPAIR                       # 24
EPS = np.float32(4e-6)               # keeps PSUM sq > 0 despite cancellation

# reduce-class per (rt, ci): ONE pattern for every row-tile so the
# per-row-tile engine load is uniform (phase-imbalance stalls the pipe).
# 6 chunks per row-tile (even count): chunks alternate psA/psB pools with
# no same-pool seam at row-tile boundaries (head-of-line blocking).
_PATTERN = ["G", "S", "A", "G", "S", "S"]
# last row-tile runs gram-free so no PE gram flush extends the tail
_PATTERN_LAST = ["A", "S", "A", "S", "S", "S"]


def _cls(rt, ci):
    return (_PATTERN_LAST if rt == RTILES - 1 else _PATTERN)[ci]


# accumulator strip layout: one f32 col per (instance, role)
_acc_slots = {}
_next_slot = [0]
for _rt in range(RTILES):
    for _ci in range(NCH):
        c = _cls(_rt, _ci)
        if c in ("S", "A"):
            _acc_slots[(_rt, _ci, "s")] = _next_slot[0]
            _next_slot[0] += 1
ACC_COLS = max(_next_slot[0], 1)

_cache = {}


def _build_nc():
    import concourse.bacc as bacc
    import concourse.mybir as mybir
    import concourse.tile as tile

    f32 = mybir.dt.float32
    bf16 = mybir.dt.bfloat16
    A = mybir.ActivationFunctionType
    OP = mybir.AluOpType

    nc = bacc.Bacc("TRN2", target_bir_lowering=False, debug=False)
    rdist = nc.dram_tensor("rdist", [ROWS_PER_CORE, N], bf16, kind="ExternalInput")
    acore = nc.dram_tensor("acore", [K, ROWS_PER_CORE], bf16, kind="ExternalInput")
    bfull = nc.dram_tensor("bfull", [K, N], bf16, kind="ExternalInput")
    gout = nc.dram_tensor("gram", [128, 128], f32, kind="ExternalOutput")
    aout = nc.dram_tensor("acc", [128, ACC_COLS], f32, kind="ExternalOutput")

    nblocks_total = sum(
        CHUNKS[ci][1] // 128
        for rt in range(RTILES) for ci in range(NCH) if _cls(rt, ci) == "G")

    with tile.TileContext(nc) as tc:
        with tc.tile_pool(name="small", bufs=1) as small, \
             tc.tile_pool(name="rpool", bufs=4) as rpool, \
             tc.tile_pool(name="ppool", bufs=8) as ppool, \
             tc.tile_pool(name="psA", bufs=1, space="PSUM") as psA, \
             tc.tile_pool(name="psB", bufs=1, space="PSUM") as psB, \
             tc.tile_pool(name="psG", bufs=1, space="PSUM") as psG:

            t_a = small.tile([K, ROWS_PER_CORE], bf16)
            t_b = small.tile([K, N], bf16)
            t_g = small.tile([128, 128], f32)
            t_acc = small.tile([128, ACC_COLS], f32)
            t_scr_d = small.tile([128, 2048], bf16)    # DVE STT#2 out sink
            t_scr_a = small.tile([128, 2048], bf16)    # ACT sqacc out sink
            t_neg1 = small.tile([128, 1], f32)
            nc.sync.dma_start(t_a[:], acore[:])
            nc.vector.memset(t_neg1[:], -1.0)

            gps = psG.tile([128, 128], f32, tag="g")

            pending = []                 # (v_tile_ap, wc) gram work queue
            blk = [0]

            def emit_gram(v_ap, wc):
                for b in range(wc // 128):
                    nc.tensor.matmul(
                        gps[:], v_ap[:, b * 128:(b + 1) * 128],
                        v_ap[:, b * 128:(b + 1) * 128],
                        start=(blk[0] == 0), stop=(blk[0] == nblocks_total - 1))
                    blk[0] += 1

            for rt in range(RTILES):
                lhsT = t_a[:, rt * 128:(rt + 1) * 128]
                if rt == 0:
                    # first row-tile: per-chunk r DMAs so the DVE chain
                    # starts as soon as the first 512KB lands
                    t_r = None
                else:
                    # r for the whole row-tile in one DMA (16KB rows)
                    t_r = rpool.tile([128, N], bf16, tag="r")
                    nc.sync.dma_start(t_r[:], rdist[rt * 128:(rt + 1) * 128, :])
                for ci, (c0, wc) in enumerate(CHUNKS):
                    cls = _cls(rt, ci)
                    if t_r is None:
                        # interleave b-chunk and r0-chunk DMAs so the first
                        # chunks of the pipeline fill as early as possible
                        nc.sync.dma_start(t_b[:, c0:c0 + wc],
                                          bfull[:, c0:c0 + wc])
                        t_rc = rpool.tile([128, wc], bf16, tag="r0")
                        nc.sync.dma_start(
                            t_rc[:], rdist[rt * 128:(rt + 1) * 128,
                                           c0:c0 + wc])
                    pool = psA if (rt * NCH + ci) % 2 == 0 else psB
                    ps = pool.tile([128, wc], f32, tag="psq")
                    for j0 in range(0, wc, 512):
                        j1 = min(j0 + 512, wc)
                        nc.tensor.matmul(
                            ps[:, j0:j1], lhsT,
                            t_b[:, c0 + j0:c0 + j1],
                            start=True, stop=True)
                    t_p = ppool.tile([128, wc], bf16, tag="pred")
                    nc.scalar.activation(t_p[:], ps[:], A.Sqrt)
                    rsl = t_rc[:] if t_r is None else t_r[:, c0:c0 + wc]
                    # all classes: w = pred * r in place (TT, 2x)
                    nc.vector.tensor_tensor(t_p[:], t_p[:], rsl, OP.mult)
                    if cls == "S":
                        # (w-2)*w with accum -> sum(w^2 - 2w); host adds M
                        sl = _acc_slots[(rt, ci, "s")]
                        nc.vector.scalar_tensor_tensor(
                            out=t_scr_d[:, :wc], in0=t_p[:], scalar=2.0,
                            in1=t_p[:], op0=OP.subtract, op1=OP.mult,
                            accum_out=t_acc[:, sl:sl + 1])
                    elif cls == "A":
                        sl = _acc_slots[(rt, ci, "s")]
                        nc.scalar.activation(
                            t_scr_a[:, :wc], t_p[:], A.Square,
                            bias=t_neg1[:], scale=1.0,
                            accum_out=t_acc[:, sl:sl + 1])
                    else:  # G
                        nc.vector.tensor_scalar(
                            out=t_p[:], in0=t_p[:],
                            scalar1=1.0, scalar2=None, op0=OP.subtract)
                        pending.append((t_p, wc))
                        if len(pending) > GRAM_LAG:
                            emit_gram(*pending.pop(0))
            while pending:
                emit_gram(*pending.pop(0))

            nc.scalar.copy(t_g[:], gps[:])
            nc.sync.dma_start(gout[:], t_g[:])
            nc.sync.dma_start(aout[:], t_acc[:])

    nc.compile()
    return nc


def _split3(v: np.ndarray):
    """Split fp32 vector into 3 bf16 terms summing to v (error ~2^-27 |v|)."""
    v = v.astype(np.float32)
    v0 = v.astype(ml_dtypes.bfloat16)
    r1 = v - v0.astype(np.float32)
    v1 = r1.astype(ml_dtypes.bfloat16)
    r2 = r1 - v1.astype(np.float32)
    v2 = r2.astype(ml_dtypes.bfloat16)
    return v0, v1, v2


def _to_np_f32(x):
    try:
        return np.ascontiguousarray(x, dtype=np.float32)
    except Exception:
        import jax
        return np.ascontiguousarray(jax.device_get(x), dtype=np.float32)


def _prep_inputs(pos: np.ndarray, dist: np.ndarray):
    pos = _to_np_f32(pos)
    dist = _to_np_f32(dist)
    assert pos.shape == (N, 2) and dist.shape == (N, N)

    # host-side prep: r = 1/d in bf16; r=0 for masked (d==0) entries so the
    # device yields exactly (0-1)^2 = 1 there (removed via nzeros below)
    zmask = dist == 0.0
    nzeros = int(np.count_nonzero(zmask))
    with np.errstate(divide="ignore"):
        r = np.where(zmask, np.float32(0.0), np.float32(1.0) / dist)
    r16 = r.astype(ml_dtypes.bfloat16)

    x = pos[:, 0].astype(np.float64)
    y = pos[:, 1].astype(np.float64)
    n = x * x + y * y
    a_full32 = np.stack([np.ones(N), n + np.float64(EPS), -2.0 * x, -2.0 * y]
                        ).astype(np.float32)          # [4, N]
    b_full32 = np.stack([n, np.ones(N), x, y]).astype(np.float32)  # [4, N]

    a0, a1, a2 = _split3(a_full32)
    b0, b1, b2 = _split3(b_full32)
    # term pairs kept: (a0,b0) (a0,b1) (a1,b0) (a0,b2) (a2,b0) (a1,b1)
    a_parts = [a0, a0, a1, a0, a2, a1]
    b_parts = [b0, b1, b0, b2, b0, b1]
    a_full = np.concatenate(a_parts, axis=0)   # [24, N] bf16
    b_full = np.concatenate(b_parts, axis=0)   # [24, N] bf16

    in_maps = []
    for c in range(NCORES):
        r0 = c * ROWS_PER_CORE
        in_maps.append({
            "rdist": np.ascontiguousarray(r16[r0:r0 + ROWS_PER_CORE, :]),
            "acore": np.ascontiguousarray(a_full[:, r0:r0 + ROWS_PER_CORE]),
            "bfull": b_full,
        })
    return in_maps, nzeros


def _combine(gram: np.ndarray, acc: np.ndarray) -> float:
    """One core's total sum((w-1)^2) over its 1024x8192 entries."""
    gram = gram.astype(np.float64)
    acc = acc.astype(np.float64)
    total = float(np.trace(gram))                    # all G-class columns
    for rt in range(RTILES):
        for ci, (c0, wc) in enumerate(CHUNKS):
            c = _cls(rt, ci)
            if c == "A":
                total += acc[:, _acc_slots[(rt, ci, "s")]].sum()
            elif c == "S":
                # accum holds sum(w^2 - 2w); (w-1)^2 = w^2 - 2w + 1
                total += acc[:, _acc_slots[(rt, ci, "s")]].sum() + 128.0 * wc
    return total


def kernel(pos: np.ndarray, dist: np.ndarray) -> np.ndarray:
    from concourse.bass_utils import run_bass_kernel_spmd

    in_maps, nzeros = _prep_inputs(pos, dist)
    if "nc" not in _cache:
        _cache["nc"] = _build_nc()
    nc = _cache["nc"]

    res = run_bass_kernel_spmd(nc, in_maps, list(range(NCORES)))
    total = -float(nzeros)
    for c in range(NCORES):
        total += _combine(res.results[c]["gram"], res.results[c]["acc"])
    return np.array(total, dtype=np.float32)
